# revision 21
# baseline (speedup 1.0000x reference)
"""ALRDLinear + KIVI(2-bit key) fused kernel for one TRN2 chip (8 NeuronCores).

    y = x @ W_B^T                    [B,S,R]
    yq = kivi_qdq(y)                 per-channel 2-bit quant along token dim,
                                     groups of 128 tokens
    out = yq @ W_A^T + b_A           [B,S,O]

Sharding: tokens (B*S) are split into 8 contiguous shards of 2048 tokens.
Quantization groups (128 tokens) never straddle shard boundaries, so the
kernel needs no collectives. Weights are replicated per core.

Precision: y is computed with a single f32r (tf32-like, 12-bit mantissa)
TensorEngine pass over host-side rne12-pre-rounded x and W_B^T. The resulting
y error (~1.5e-4 rms) shifts a small fraction of KIVI round() decisions; the
measured end-to-end relative error is ~1.3e-2, within the 2e-2 budget.
Host pre-rounding makes the engine's internal f32r rounding an identity, so
device results match the numpy model. MM2 runs in bf16 (smooth error).
"""
import numpy as np
from contextlib import ExitStack

import concourse.bass as bass
import concourse.tile as tile
from concourse import bacc, mybir
from concourse.alu_op_type import AluOpType
from concourse.bass_utils import run_bass_kernel_spmd

F32 = mybir.dt.float32
F32R = mybir.dt.float32r
BF16 = mybir.dt.bfloat16
F16 = mybir.dt.float16
FP8 = mybir.dt.float8e4
MAGIC = float(np.float32(2.0 ** 23))
AF = mybir.ActivationFunctionType

N_CORES = 8
B, S, D, R, O = 4, 4096, 4096, 512, 4096
TOK = B * S // N_CORES


def _build_nc(TOK=TOK, D=D, R=R, O=O, BLK=512, GRP=128,
              xt_bufs=8, yq_bufs=8, psum_y_bufs=6, psum_o_bufs=2,
              out_bufs=3, ysb_bufs=4):
    P = 128
    DC = D // P
    RB = R // P
    NB = TOK // BLK
    GPB = BLK // GRP
    OCW = 512
    OC = O // OCW
    TKC = BLK // P
    assert GRP == P

    nc = bacc.Bacc()
    NB_ = TOK // BLK
    PK = BLK + BLK // 2
    xpk = nc.declare_dram_parameter("xpk", [D, NB_, PK], F16, isOutput=False)
    wpk = nc.declare_dram_parameter("wpk", [D, R + R // 2], F16, isOutput=False)
    wat = nc.declare_dram_parameter("wat", [R, O], BF16, isOutput=False)
    out = nc.declare_dram_parameter("out", [TOK, O], BF16, isOutput=True)

    with tile.TileContext(nc) as tc, ExitStack() as ctx:
        pool_w = ctx.enter_context(tc.tile_pool(name="w_persist", bufs=1))
        pool_xt = ctx.enter_context(tc.tile_pool(name="xt", bufs=xt_bufs))
        pool_x3 = ctx.enter_context(tc.tile_pool(name="x3", bufs=6))
        pool_t = ctx.enter_context(tc.tile_pool(name="tq", bufs=4))
        pool_ysb = ctx.enter_context(tc.tile_pool(name="ysb", bufs=ysb_bufs))
        pool_yq = ctx.enter_context(tc.tile_pool(name="yq", bufs=yq_bufs))
        pool_sm = ctx.enter_context(tc.tile_pool(name="small", bufs=6))
        pool_out = ctx.enter_context(tc.tile_pool(name="outsb", bufs=out_bufs))
        pool_py = ctx.enter_context(
            tc.tile_pool(name="psum_y", bufs=psum_y_bufs, space="PSUM"))
        pool_po = ctx.enter_context(
            tc.tile_pool(name="psum_o", bufs=psum_o_bufs, space="PSUM"))

        wr_sb = [None] * DC
        wat_sb = [None] * RB

        def load_wr_chunk(c):
            w3 = pool_x3.tile([P, R + R // 2], F16, tag="w3", name=f"w3_{c}")
            nc.scalar.dma_start(out=w3, in_=wpk[c * P:(c + 1) * P, :])
            w_t = pool_w.tile([P, R], F32R, tag=f"wr{c}", name=f"wr_{c}")
            nc.vector.scalar_tensor_tensor(w_t, w3[:, R:].bitcast(FP8),
                                           2.0 ** -15, w3[:, :R],
                                           AluOpType.mult, AluOpType.add)
            wr_sb[c] = w_t

        def alloc_mm2_weights():
            for rb in range(RB):
                wat_sb[rb] = pool_w.tile([P, O], BF16, tag=f"wat{rb}",
                                         name=f"wat_{rb}")

        def load_wat_slice(i):
            # i in 0..RB*4-1: load [P, O/4] slice of one wat row-block
            rb, s = divmod(i, 4)
            o0, o1 = s * (O // 4), (s + 1) * (O // 4)
            nc.gpsimd.dma_start(out=wat_sb[rb][:, o0:o1],
                                in_=wat[rb * P:(rb + 1) * P, o0:o1])

        def emit_warmup():
            # ~28 tiny matmuls on zeros while the first DMAs land: keeps the
            # PE HAM activity window busy so real matmuls start at full clock
            w0 = pool_sm.tile([P, P], F32, tag="warm_w")
            nc.vector.memset(w0[:], 0.0)
            ps = pool_py.tile([P, GPB, GRP], F32, tag="py", name="py_warm")
            for i in range(28):
                nc.tensor.matmul(ps[:, 0, :], w0[:], w0[:], start=True,
                                 stop=True)

        def emit_mm1(b, mm2_groups):
            tok0 = b * BLK
            py = [pool_py.tile([P, GPB, GRP], F32, tag="py", name=f"py_{b}_{rb}")
                  for rb in range(RB)]
            for c in range(DC):
                if b == 0:
                    load_wr_chunk(c)
                    if 4 <= c < 20:
                        load_wat_slice(c - 4)
                x3 = pool_x3.tile([P, PK], F16, tag="x3")
                nc.sync.dma_start(out=x3, in_=xpk[c * P:(c + 1) * P, b, :])
                x_in = pool_xt.tile([P, BLK], F32R, tag="x_in")
                nc.vector.scalar_tensor_tensor(x_in, x3[:, BLK:].bitcast(FP8),
                                               2.0 ** -15, x3[:, :BLK],
                                               AluOpType.mult, AluOpType.add)
                first = c == 0
                last = c == DC - 1
                for rb in range(RB):
                    w0, w1 = rb * P, (rb + 1) * P
                    nc.tensor.matmul(py[rb][:], wr_sb[c][:, w0:w1], x_in,
                                     start=first, stop=last)
                # interleave prev block's MM2 starting at chunk 8 (2 groups per
                # chunk for c=8..15, then 1 per chunk) so MM2's wat dependency
                # never head-of-line-blocks the tensor queue at block start
                if c >= 8 and mm2_groups:
                    emit_mm2_group(*mm2_groups.pop(0))
                    if c < 16 and mm2_groups:
                        emit_mm2_group(*mm2_groups.pop(0))
            return py

        def emit_quant_fine(b, py):
            """Tail quant: coarse per-rb copies/reduces (one op covers all 4
            groups), small ops split DVE/GPSIMD by rb parity, then per-group
            activations emitted group-major so yq for token-tile 0 lands
            ~2us after mm1 ends and the tail MM2 (tk-major) starts early."""
            ysb_l = [pool_ysb.tile([P, GPB, GRP], F32, tag="ysb",
                                   name=f"ysbf_{rb}") for rb in range(RB)]
            t_l = [pool_t.tile([P, GPB, GRP], F32, tag="t", name=f"tf_{rb}")
                   for rb in range(RB)]
            yq_l = [pool_yq.tile([P, GPB, GRP], BF16, tag="yq",
                                 name=f"yqf_{rb}") for rb in range(RB)]
            mn_l = [pool_sm.tile([P, GPB], F32, tag="mn", name=f"mnf_{rb}")
                    for rb in range(RB)]
            mx_l = [pool_sm.tile([P, GPB], F32, tag="mx", name=f"mxf_{rb}")
                    for rb in range(RB)]
            sc_l = [pool_sm.tile([P, GPB], F32, tag="scale", name=f"scf_{rb}")
                    for rb in range(RB)]
            rs_l = [pool_sm.tile([P, GPB], F32, tag="rscale", name=f"rsf_{rb}")
                    for rb in range(RB)]
            nb_l = [pool_sm.tile([P, GPB], F32, tag="nbias", name=f"nbf_{rb}")
                    for rb in range(RB)]
            for rb in range(RB):
                ve = nc.vector
                src_t = ysb_l[rb]
                if rb % 2 == 0:
                    nc.vector.tensor_copy(out=src_t, in_=py[rb][:])
                else:
                    nc.scalar.activation(out=src_t, in_=py[rb][:],
                                         func=AF.Identity)
                nc.vector.tensor_reduce(mn_l[rb], src_t[:],
                                        mybir.AxisListType.X, AluOpType.min)
                nc.vector.tensor_reduce(mx_l[rb], src_t[:],
                                        mybir.AxisListType.X, AluOpType.max)
                ve.tensor_tensor(sc_l[rb], mx_l[rb], mn_l[rb],
                                 AluOpType.subtract)
                ve.tensor_scalar(sc_l[rb], sc_l[rb], 1.0 / 3.0, 1e-8,
                                 AluOpType.mult, AluOpType.max)
                nc.vector.reciprocal(out=rs_l[rb], in_=sc_l[rb])
                ve.scalar_tensor_tensor(nb_l[rb], mn_l[rb], -1.0, rs_l[rb],
                                        AluOpType.mult, AluOpType.mult)
            for g in range(GPB):
                for rb in range(RB):
                    ve = nc.vector
                    nc.scalar.activation(out=t_l[rb][:, g, :],
                                         in_=ysb_l[rb][:, g, :],
                                         func=AF.Identity,
                                         bias=nb_l[rb][:, g:g + 1],
                                         scale=rs_l[rb][:, g:g + 1])
                    ve.tensor_scalar(t_l[rb][:, g, :], t_l[rb][:, g, :],
                                     MAGIC, MAGIC,
                                     AluOpType.add, AluOpType.subtract)
                    nc.scalar.activation(out=yq_l[rb][:, g, :],
                                         in_=t_l[rb][:, g, :],
                                         func=AF.Identity,
                                         bias=mn_l[rb][:, g:g + 1],
                                         scale=sc_l[rb][:, g:g + 1])
            return yq_l

        def emit_quant(b, py):
            yq = []
            for rb in range(RB):
                ysb = pool_ysb.tile([P, GPB, GRP], F32, tag="ysb")
                if rb % 2 == 0:
                    nc.vector.tensor_copy(out=ysb, in_=py[rb][:])
                else:
                    nc.scalar.activation(out=ysb, in_=py[rb][:], func=AF.Identity)
                src = ysb
                mn = pool_sm.tile([P, GPB], F32, tag="mn")
                mx = pool_sm.tile([P, GPB], F32, tag="mx")
                nc.vector.tensor_reduce(mn, src[:], mybir.AxisListType.X, AluOpType.min)
                nc.vector.tensor_reduce(mx, src[:], mybir.AxisListType.X, AluOpType.max)
                diff = pool_sm.tile([P, GPB], F32, tag="diff")
                nc.vector.tensor_tensor(diff, mx, mn, AluOpType.subtract)
                scale = pool_sm.tile([P, GPB], F32, tag="scale")
                nc.vector.tensor_scalar(scale, diff, 1.0 / 3.0, 1e-8,
                                        AluOpType.mult, AluOpType.max)
                rscale = pool_sm.tile([P, GPB], F32, tag="rscale")
                nc.vector.reciprocal(out=rscale, in_=scale)
                nbias = pool_sm.tile([P, GPB], F32, tag="nbias")
                nc.vector.scalar_tensor_tensor(nbias, mn, -1.0, rscale,
                                               AluOpType.mult, AluOpType.mult)
                t = pool_t.tile([P, GPB, GRP], F32, tag="t")
                for g in range(GPB):
                    nc.scalar.activation(out=t[:, g, :], in_=src[:, g, :],
                                         func=AF.Identity,
                                         bias=nbias[:, g:g + 1],
                                         scale=rscale[:, g:g + 1])
                nc.vector.tensor_scalar(t[:], t[:], MAGIC, MAGIC,
                                        AluOpType.add, AluOpType.subtract)
                yq_t = pool_yq.tile([P, GPB, GRP], BF16, tag="yq")
                for g in range(GPB):
                    nc.scalar.activation(out=yq_t[:, g, :], in_=t[:, g, :],
                                         func=AF.Identity,
                                         bias=mn[:, g:g + 1],
                                         scale=scale[:, g:g + 1])
                yq.append(yq_t)
            return yq

        ob_cur = [None]

        def emit_mm2_group(b, yq, oc, tk):
            tok0 = b * BLK
            o0, o1 = oc * OCW, (oc + 1) * OCW
            po = pool_po.tile([P, OCW], F32, tag="po", name=f"po_{b}_{oc}_{tk}")
            for rb in range(RB):
                nc.tensor.matmul(po[:], yq[rb][:, tk, :], wat_sb[rb][:, o0:o1],
                                 start=(rb == 0), stop=(rb == RB - 1))
            if oc == 0:
                ob_cur[0] = pool_out.tile([P, O], BF16, tag="ob",
                                          name=f"ob_{b}_{tk}")
            ob = ob_cur[0]
            nc.scalar.activation(out=ob[:, o0:o1], in_=po[:], func=AF.Identity)
            if oc == OC - 1:
                nc.scalar.dma_start(
                    out=out[tok0 + tk * P: tok0 + (tk + 1) * P, :], in_=ob)

        def mm2_group_list(b, yq):
            return [(b, yq, oc, tk) for tk in range(TKC) for oc in range(OC)]

        emit_warmup()
        alloc_mm2_weights()
        prev = None
        for b in range(NB):
            py = emit_mm1(b, mm2_group_list(b - 1, prev) if prev is not None else [])
            prev = (emit_quant_fine(b, py) if b == NB - 1
                    else emit_quant(b, py))
        for g in mm2_group_list(NB - 1, prev):
            emit_mm2_group(*g)
    nc.finalize()
    return nc


def _pack_hi_lo(a):
    """Pack f32 [D, N] as fp16(a) followed by fp8((a - fp16)*2^15) bytes,
    in one fp16-typed [D, N + N//2] array (single contiguous DMA line)."""
    import ml_dtypes
    Dd, N = a.shape
    hi = a.astype(np.float16)
    resid = (a - hi.astype(np.float32)) * np.float32(2.0 ** 15)
    lo8 = np.asarray(resid, dtype=ml_dtypes.float8_e4m3fn)
    pack = np.empty((Dd, N + N // 2), dtype=np.float16)
    pu8 = pack.view(np.uint8)
    pu8[:, :2 * N] = hi.view(np.uint8)
    pu8[:, 2 * N:] = lo8.view(np.uint8)
    return pack


def _make_in_maps(input, W_B, W_A, b_A, BLK=512):
    import ml_dtypes
    x = np.ascontiguousarray(np.asarray(input, dtype=np.float32))
    W_B = np.asarray(W_B, dtype=np.float32)
    W_A = np.asarray(W_A, dtype=np.float32)
    b_A = np.asarray(b_A, dtype=np.float32)
    Bi, Si, Di = x.shape

    toks = Bi * Si
    tok_pc = toks // N_CORES
    xf = np.ascontiguousarray(x.reshape(toks, Di))
    wbt = np.ascontiguousarray(W_B.T).astype(np.float32)
    wpk = _pack_hi_lo(wbt)
    wat = np.ascontiguousarray(W_A.T).astype(ml_dtypes.bfloat16)
    NB_ = tok_pc // BLK
    PK = BLK + BLK // 2
    in_maps = []
    for c in range(N_CORES):
        shard = np.ascontiguousarray(xf[c * tok_pc:(c + 1) * tok_pc].T)
        xpk = np.empty((Di, NB_, PK), dtype=np.float16)
        xu8 = xpk.view(np.uint8)
        for b in range(NB_):
            blk = _pack_hi_lo(shard[:, b * BLK:(b + 1) * BLK])
            xu8[:, b, :] = blk.view(np.uint8)
        in_maps.append({"xpk": xpk, "wpk": wpk, "wat": wat})
    return in_maps, (Bi, Si, Di, W_B.shape[0], W_A.shape[0], tok_pc)


def kernel(input, W_B, W_A, b_A):
    in_maps, (Bi, Si, Di, Ri, Oi, tok_pc) = _make_in_maps(input, W_B, W_A, b_A)
    nc = _build_nc(TOK=tok_pc, D=Di, R=Ri, O=Oi)
    res = run_bass_kernel_spmd(nc, in_maps, core_ids=list(range(N_CORES)),
                               trace=False)
    b_A = np.asarray(b_A, dtype=np.float32)
    out = np.concatenate([np.asarray(res.results[c]["out"]).astype(np.float32)
                          for c in range(N_CORES)], axis=0)
    out += b_A
    return out.reshape(Bi, Si, Oi)


# revision 23
# speedup vs baseline: 1.0090x; 1.0090x over previous
"""ALRDLinear + KIVI(2-bit key) fused kernel for one TRN2 chip (8 NeuronCores).

    y = x @ W_B^T                    [B,S,R]
    yq = kivi_qdq(y)                 per-channel 2-bit quant along token dim,
                                     groups of 128 tokens
    out = yq @ W_A^T + b_A           [B,S,O]

Sharding: tokens (B*S) are split into 8 contiguous shards of 2048 tokens.
Quantization groups (128 tokens) never straddle shard boundaries, so the
kernel needs no collectives. Weights are replicated per core.

Precision: y is computed with a single f32r (tf32-like, 12-bit mantissa)
TensorEngine pass over host-side rne12-pre-rounded x and W_B^T. The resulting
y error (~1.5e-4 rms) shifts a small fraction of KIVI round() decisions; the
measured end-to-end relative error is ~1.3e-2, within the 2e-2 budget.
Host pre-rounding makes the engine's internal f32r rounding an identity, so
device results match the numpy model. MM2 runs in bf16 (smooth error).
"""
import numpy as np
from contextlib import ExitStack

import concourse.bass as bass
import concourse.tile as tile
from concourse import bacc, mybir
from concourse.alu_op_type import AluOpType
from concourse.bass_utils import run_bass_kernel_spmd

F32 = mybir.dt.float32
F32R = mybir.dt.float32r
BF16 = mybir.dt.bfloat16
F16 = mybir.dt.float16
FP8 = mybir.dt.float8e4
MAGIC = float(np.float32(2.0 ** 23))
AF = mybir.ActivationFunctionType

N_CORES = 8
B, S, D, R, O = 4, 4096, 4096, 512, 4096
TOK = B * S // N_CORES


def _build_nc(TOK=TOK, D=D, R=R, O=O, BLK=512, GRP=128,
              xt_bufs=8, yq_bufs=8, psum_y_bufs=6, psum_o_bufs=2,
              out_bufs=3, ysb_bufs=4):
    P = 128
    DC = D // P
    RB = R // P
    NB = TOK // BLK
    GPB = BLK // GRP
    OCW = 512
    OC = O // OCW
    TKC = BLK // P
    assert GRP == P

    nc = bacc.Bacc()
    NB_ = TOK // BLK
    PK = BLK + BLK // 2
    xpk = nc.declare_dram_parameter("xpk", [D, NB_, PK], F16, isOutput=False)
    wpk = nc.declare_dram_parameter("wpk", [D, R + R // 2], F16, isOutput=False)
    wat = nc.declare_dram_parameter("wat", [R, O], BF16, isOutput=False)
    out = nc.declare_dram_parameter("out", [TOK, O], BF16, isOutput=True)

    with tile.TileContext(nc) as tc, ExitStack() as ctx:
        pool_w = ctx.enter_context(tc.tile_pool(name="w_persist", bufs=1))
        pool_xt = ctx.enter_context(tc.tile_pool(name="xt", bufs=xt_bufs))
        pool_x3 = ctx.enter_context(tc.tile_pool(name="x3", bufs=6))
        pool_t = ctx.enter_context(tc.tile_pool(name="tq", bufs=4))
        pool_ysb = ctx.enter_context(tc.tile_pool(name="ysb", bufs=ysb_bufs))
        pool_yq = ctx.enter_context(tc.tile_pool(name="yq", bufs=yq_bufs))
        pool_sm = ctx.enter_context(tc.tile_pool(name="small", bufs=6))
        pool_out = ctx.enter_context(tc.tile_pool(name="outsb", bufs=out_bufs))
        pool_py = ctx.enter_context(
            tc.tile_pool(name="psum_y", bufs=psum_y_bufs, space="PSUM"))
        pool_po = ctx.enter_context(
            tc.tile_pool(name="psum_o", bufs=psum_o_bufs, space="PSUM"))

        wr_sb = [None] * DC
        wat_sb = [None] * RB

        def load_wr_chunk(c):
            w3 = pool_x3.tile([P, R + R // 2], F16, tag="w3", name=f"w3_{c}")
            nc.scalar.dma_start(out=w3, in_=wpk[c * P:(c + 1) * P, :])
            w_t = pool_w.tile([P, R], F32R, tag=f"wr{c}", name=f"wr_{c}")
            nc.vector.scalar_tensor_tensor(w_t, w3[:, R:].bitcast(FP8),
                                           2.0 ** -15, w3[:, :R],
                                           AluOpType.mult, AluOpType.add)
            wr_sb[c] = w_t

        def alloc_mm2_weights():
            for rb in range(RB):
                wat_sb[rb] = pool_w.tile([P, O], BF16, tag=f"wat{rb}",
                                         name=f"wat_{rb}")

        def load_wat_slice(i):
            # i in 0..RB*4-1: load [P, O/4] slice of one wat row-block
            rb, s = divmod(i, 4)
            o0, o1 = s * (O // 4), (s + 1) * (O // 4)
            nc.gpsimd.dma_start(out=wat_sb[rb][:, o0:o1],
                                in_=wat[rb * P:(rb + 1) * P, o0:o1])

        def emit_warmup():
            # ~28 tiny matmuls on zeros while the first DMAs land: keeps the
            # PE HAM activity window busy so real matmuls start at full clock
            w0 = pool_sm.tile([P, P], F32, tag="warm_w")
            nc.vector.memset(w0[:], 0.0)
            ps = pool_py.tile([P, GPB, GRP], F32, tag="py", name="py_warm")
            for i in range(28):
                nc.tensor.matmul(ps[:, 0, :], w0[:], w0[:], start=True,
                                 stop=True)

        def emit_mm1(b, mm2_groups, quant_units):
            tok0 = b * BLK
            py = [pool_py.tile([P, GPB, GRP], F32, tag="py", name=f"py_{b}_{rb}")
                  for rb in range(RB)]
            for c in range(DC):
                if b == 0:
                    load_wr_chunk(c)
                    if 4 <= c < 20:
                        load_wat_slice(c - 4)
                x3 = pool_x3.tile([P, PK], F16, tag="x3")
                nc.sync.dma_start(out=x3, in_=xpk[c * P:(c + 1) * P, b, :])
                x_in = pool_xt.tile([P, BLK], F32R, tag="x_in")
                nc.vector.scalar_tensor_tensor(x_in, x3[:, BLK:].bitcast(FP8),
                                               2.0 ** -15, x3[:, :BLK],
                                               AluOpType.mult, AluOpType.add)
                first = c == 0
                last = c == DC - 1
                for rb in range(RB):
                    w0, w1 = rb * P, (rb + 1) * P
                    nc.tensor.matmul(py[rb][:], wr_sb[c][:, w0:w1], x_in,
                                     start=first, stop=last)
                # prev block's quant chains drip in at c=2,4,6,8 so they never
                # monopolize the DVE queue ahead of this block's x recon
                if c in (2, 4, 6, 8) and quant_units:
                    quant_units.pop(0)()
                # prev block's MM2 from chunk 12 (2/chunk until 23, then 1)
                if c >= 12 and mm2_groups:
                    emit_mm2_group(*mm2_groups.pop(0))
                    if c < 24 and mm2_groups:
                        emit_mm2_group(*mm2_groups.pop(0))
            return py

        def emit_quant_fine(b, py):
            """Tail quant: coarse per-rb copies/reduces (one op covers all 4
            groups), small ops split DVE/GPSIMD by rb parity, then per-group
            activations emitted group-major so yq for token-tile 0 lands
            ~2us after mm1 ends and the tail MM2 (tk-major) starts early."""
            ysb_l = [pool_ysb.tile([P, GPB, GRP], F32, tag="ysb",
                                   name=f"ysbf_{rb}") for rb in range(RB)]
            t_l = [pool_t.tile([P, GPB, GRP], F32, tag="t", name=f"tf_{rb}")
                   for rb in range(RB)]
            yq_l = [pool_yq.tile([P, GPB, GRP], BF16, tag="yq",
                                 name=f"yqf_{rb}") for rb in range(RB)]
            mn_l = [pool_sm.tile([P, GPB], F32, tag="mn", name=f"mnf_{rb}")
                    for rb in range(RB)]
            mx_l = [pool_sm.tile([P, GPB], F32, tag="mx", name=f"mxf_{rb}")
                    for rb in range(RB)]
            sc_l = [pool_sm.tile([P, GPB], F32, tag="scale", name=f"scf_{rb}")
                    for rb in range(RB)]
            rs_l = [pool_sm.tile([P, GPB], F32, tag="rscale", name=f"rsf_{rb}")
                    for rb in range(RB)]
            nb_l = [pool_sm.tile([P, GPB], F32, tag="nbias", name=f"nbf_{rb}")
                    for rb in range(RB)]
            for rb in range(RB):
                ve = nc.vector
                src_t = ysb_l[rb]
                if rb % 2 == 0:
                    nc.vector.tensor_copy(out=src_t, in_=py[rb][:])
                else:
                    nc.scalar.activation(out=src_t, in_=py[rb][:],
                                         func=AF.Identity)
                nc.vector.tensor_reduce(mn_l[rb], src_t[:],
                                        mybir.AxisListType.X, AluOpType.min)
                nc.vector.tensor_reduce(mx_l[rb], src_t[:],
                                        mybir.AxisListType.X, AluOpType.max)
                ve.tensor_tensor(sc_l[rb], mx_l[rb], mn_l[rb],
                                 AluOpType.subtract)
                ve.tensor_scalar(sc_l[rb], sc_l[rb], 1.0 / 3.0, 1e-8,
                                 AluOpType.mult, AluOpType.max)
                nc.vector.reciprocal(out=rs_l[rb], in_=sc_l[rb])
                ve.scalar_tensor_tensor(nb_l[rb], mn_l[rb], -1.0, rs_l[rb],
                                        AluOpType.mult, AluOpType.mult)
            for g in range(GPB):
                for rb in range(RB):
                    ve = nc.vector
                    nc.scalar.activation(out=t_l[rb][:, g, :],
                                         in_=ysb_l[rb][:, g, :],
                                         func=AF.Identity,
                                         bias=nb_l[rb][:, g:g + 1],
                                         scale=rs_l[rb][:, g:g + 1])
                    ve.tensor_scalar(t_l[rb][:, g, :], t_l[rb][:, g, :],
                                     MAGIC, MAGIC,
                                     AluOpType.add, AluOpType.subtract)
                    nc.scalar.activation(out=yq_l[rb][:, g, :],
                                         in_=t_l[rb][:, g, :],
                                         func=AF.Identity,
                                         bias=mn_l[rb][:, g:g + 1],
                                         scale=sc_l[rb][:, g:g + 1])
            return yq_l

        def emit_quant(b, py):
            """Copy PSUM out now (releases py for the next block); return the
            rest of each rb's chain as closures to drip into the next block's
            chunk loop so they never monopolize the DVE queue."""
            ysb_l = []
            for rb in range(RB):
                ysb = pool_ysb.tile([P, GPB, GRP], F32, tag="ysb",
                                    name=f"ysb_{b}_{rb}")
                if rb % 2 == 0:
                    nc.vector.tensor_copy(out=ysb, in_=py[rb][:])
                else:
                    nc.scalar.activation(out=ysb, in_=py[rb][:], func=AF.Identity)
                ysb_l.append(ysb)
            yq = [pool_yq.tile([P, GPB, GRP], BF16, tag="yq",
                               name=f"yq_{b}_{rb}") for rb in range(RB)]

            def make_unit(rb):
                def unit():
                    src = ysb_l[rb]
                    yq_t = yq[rb]
                    mn = pool_sm.tile([P, GPB], F32, tag="mn", name=f"mn_{b}_{rb}")
                    mx = pool_sm.tile([P, GPB], F32, tag="mx", name=f"mx_{b}_{rb}")
                    nc.vector.tensor_reduce(mn, src[:], mybir.AxisListType.X,
                                            AluOpType.min)
                    nc.vector.tensor_reduce(mx, src[:], mybir.AxisListType.X,
                                            AluOpType.max)
                    scale = pool_sm.tile([P, GPB], F32, tag="scale",
                                         name=f"sc_{b}_{rb}")
                    nc.vector.tensor_tensor(scale, mx, mn, AluOpType.subtract)
                    nc.vector.tensor_scalar(scale, scale, 1.0 / 3.0, 1e-8,
                                            AluOpType.mult, AluOpType.max)
                    rscale = pool_sm.tile([P, GPB], F32, tag="rscale",
                                          name=f"rs_{b}_{rb}")
                    nc.vector.reciprocal(out=rscale, in_=scale)
                    nbias = pool_sm.tile([P, GPB], F32, tag="nbias",
                                         name=f"nb_{b}_{rb}")
                    nc.vector.scalar_tensor_tensor(nbias, mn, -1.0, rscale,
                                                   AluOpType.mult, AluOpType.mult)
                    t = pool_t.tile([P, GPB, GRP], F32, tag="t",
                                    name=f"t_{b}_{rb}")
                    for g in range(GPB):
                        nc.scalar.activation(out=t[:, g, :], in_=src[:, g, :],
                                             func=AF.Identity,
                                             bias=nbias[:, g:g + 1],
                                             scale=rscale[:, g:g + 1])
                    nc.vector.tensor_scalar(t[:], t[:], MAGIC, MAGIC,
                                            AluOpType.add, AluOpType.subtract)
                    for g in range(GPB):
                        nc.scalar.activation(out=yq_t[:, g, :], in_=t[:, g, :],
                                             func=AF.Identity,
                                             bias=mn[:, g:g + 1],
                                             scale=scale[:, g:g + 1])
                return unit

            units = [make_unit(rb) for rb in range(RB)]
            return yq, units

        def emit_quant_tail(b, py):
            """Last block: no next block to hide latency in, and no need to
            free PSUM — reduce/activate straight from PSUM, rb-staggered so
            the first tail MM2 matmul can start ~2us after mm1 ends."""
            yq_l = [pool_yq.tile([P, GPB, GRP], BF16, tag="yq",
                                 name=f"yqt_{rb}") for rb in range(RB)]
            for rb in range(RB):
                mn = pool_sm.tile([P, GPB], F32, tag="mn", name=f"mnt_{rb}")
                mx = pool_sm.tile([P, GPB], F32, tag="mx", name=f"mxt_{rb}")
                nc.vector.tensor_reduce(mn, py[rb][:], mybir.AxisListType.X,
                                        AluOpType.min)
                nc.vector.tensor_reduce(mx, py[rb][:], mybir.AxisListType.X,
                                        AluOpType.max)
                scale = pool_sm.tile([P, GPB], F32, tag="scale",
                                     name=f"sct_{rb}")
                nc.vector.tensor_tensor(scale, mx, mn, AluOpType.subtract)
                nc.vector.tensor_scalar(scale, scale, 1.0 / 3.0, 1e-8,
                                        AluOpType.mult, AluOpType.max)
                rscale = pool_sm.tile([P, GPB], F32, tag="rscale",
                                      name=f"rst_{rb}")
                nc.vector.reciprocal(out=rscale, in_=scale)
                nbias = pool_sm.tile([P, GPB], F32, tag="nbias",
                                     name=f"nbt_{rb}")
                nc.vector.scalar_tensor_tensor(nbias, mn, -1.0, rscale,
                                               AluOpType.mult, AluOpType.mult)
                t = pool_t.tile([P, GPB, GRP], F32, tag="t", name=f"tt_{rb}")
                for g in range(GPB):
                    nc.scalar.activation(out=t[:, g, :], in_=py[rb][:, g, :],
                                         func=AF.Identity,
                                         bias=nbias[:, g:g + 1],
                                         scale=rscale[:, g:g + 1])
                nc.vector.tensor_scalar(t[:], t[:], MAGIC, MAGIC,
                                        AluOpType.add, AluOpType.subtract)
                for g in range(GPB):
                    nc.scalar.activation(out=yq_l[rb][:, g, :], in_=t[:, g, :],
                                         func=AF.Identity,
                                         bias=mn[:, g:g + 1],
                                         scale=scale[:, g:g + 1])
            return yq_l

        ob_cur = [None]

        def emit_mm2_group(b, yq, oc, tk):
            tok0 = b * BLK
            o0, o1 = oc * OCW, (oc + 1) * OCW
            po = pool_po.tile([P, OCW], F32, tag="po", name=f"po_{b}_{oc}_{tk}")
            for rb in range(RB):
                nc.tensor.matmul(po[:], yq[rb][:, tk, :], wat_sb[rb][:, o0:o1],
                                 start=(rb == 0), stop=(rb == RB - 1))
            if oc == 0:
                ob_cur[0] = pool_out.tile([P, O], BF16, tag="ob",
                                          name=f"ob_{b}_{tk}")
            ob = ob_cur[0]
            nc.scalar.activation(out=ob[:, o0:o1], in_=po[:], func=AF.Identity)
            if oc == OC - 1:
                nc.scalar.dma_start(
                    out=out[tok0 + tk * P: tok0 + (tk + 1) * P, :], in_=ob)

        def mm2_group_list(b, yq):
            return [(b, yq, oc, tk) for tk in range(TKC) for oc in range(OC)]

        emit_warmup()
        alloc_mm2_weights()
        prev_yq = None
        prev_units = []
        for b in range(NB):
            groups = mm2_group_list(b - 1, prev_yq) if prev_yq is not None else []
            py = emit_mm1(b, groups, prev_units)
            if b == NB - 1:
                prev_yq = emit_quant_tail(b, py)
                prev_units = []
            else:
                prev_yq, prev_units = emit_quant(b, py)
        for g in mm2_group_list(NB - 1, prev_yq):
            emit_mm2_group(*g)
    nc.finalize()
    return nc


def _pack_hi_lo(a):
    """Pack f32 [D, N] as fp16(a) followed by fp8((a - fp16)*2^15) bytes,
    in one fp16-typed [D, N + N//2] array (single contiguous DMA line)."""
    import ml_dtypes
    Dd, N = a.shape
    hi = a.astype(np.float16)
    resid = (a - hi.astype(np.float32)) * np.float32(2.0 ** 15)
    lo8 = np.asarray(resid, dtype=ml_dtypes.float8_e4m3fn)
    pack = np.empty((Dd, N + N // 2), dtype=np.float16)
    pu8 = pack.view(np.uint8)
    pu8[:, :2 * N] = hi.view(np.uint8)
    pu8[:, 2 * N:] = lo8.view(np.uint8)
    return pack


def _make_in_maps(input, W_B, W_A, b_A, BLK=512):
    import ml_dtypes
    x = np.ascontiguousarray(np.asarray(input, dtype=np.float32))
    W_B = np.asarray(W_B, dtype=np.float32)
    W_A = np.asarray(W_A, dtype=np.float32)
    b_A = np.asarray(b_A, dtype=np.float32)
    Bi, Si, Di = x.shape

    toks = Bi * Si
    tok_pc = toks // N_CORES
    xf = np.ascontiguousarray(x.reshape(toks, Di))
    wbt = np.ascontiguousarray(W_B.T).astype(np.float32)
    wpk = _pack_hi_lo(wbt)
    wat = np.ascontiguousarray(W_A.T).astype(ml_dtypes.bfloat16)
    NB_ = tok_pc // BLK
    PK = BLK + BLK // 2
    in_maps = []
    for c in range(N_CORES):
        shard = np.ascontiguousarray(xf[c * tok_pc:(c + 1) * tok_pc].T)
        xpk = np.empty((Di, NB_, PK), dtype=np.float16)
        xu8 = xpk.view(np.uint8)
        for b in range(NB_):
            blk = _pack_hi_lo(shard[:, b * BLK:(b + 1) * BLK])
            xu8[:, b, :] = blk.view(np.uint8)
        in_maps.append({"xpk": xpk, "wpk": wpk, "wat": wat})
    return in_maps, (Bi, Si, Di, W_B.shape[0], W_A.shape[0], tok_pc)


def kernel(input, W_B, W_A, b_A):
    in_maps, (Bi, Si, Di, Ri, Oi, tok_pc) = _make_in_maps(input, W_B, W_A, b_A)
    nc = _build_nc(TOK=tok_pc, D=Di, R=Ri, O=Oi)
    res = run_bass_kernel_spmd(nc, in_maps, core_ids=list(range(N_CORES)),
                               trace=False)
    b_A = np.asarray(b_A, dtype=np.float32)
    out = np.concatenate([np.asarray(res.results[c]["out"]).astype(np.float32)
                          for c in range(N_CORES)], axis=0)
    out += b_A
    return out.reshape(Bi, Si, Oi)
